# revision 14
# baseline (speedup 1.0000x reference)
"""ClusterGCN layer on 8 TRN2 NeuronCores.

Math per cluster c (only intra-cluster edges matter):
    Y_c = D^-1/2 (A_c + I) D^-1/2 X_c W + b,  D = intra-degree + 1.
Host pre-scales X rows by dis = rsqrt(deg) and ships integer edge
counts At (exact in fp8e4 for counts <= 16); device computes
    xws = Xs @ W                      (nodes on partitions)
    Z^T[f, d] = sum_s xws[s, f] * At_c[s, d]
host applies dis[d] + bias on gather and passes through clusters with
no intra edges.

Device pipeline per cluster (all matmuls fp16 on the PE):
  step1: 2 PSUM banks, each accumulating two 128-node s-tiles of
         Xs @ W; drained by scalar (bank 0) and vector (bank 1) casts.
  step2: fc x d-chunk matmuls vs fp8 At (moving), drained by vector.
DRAM layouts are partition-major so every DMA row is one long run:
  XT [P, kc, cpc*cap]   AT [P, cpc, sch*dcap]   YT [cpc, P, fc, dcap]
Queues: sync = W/X/At loads (groups of 1,1,2,2,3,4 clusters so compute
starts early), gpsimd = YT stores (stores never block upcoming loads).
Five dummy matmuls on a zeroed scratch tile warm the PE out of its
low/mid p-states (0.65/1.2 GHz) while the first loads are in flight.
"""

import numpy as np

N_CORES = 8
N_CLUSTERS = 100
P = 128
N_WARM = 7

_prog_cache: dict = {}


def _build_program(cpc: int, cap: int, dcap: int, in_c: int, f_out: int,
                   a_fp8: bool):
    import concourse.mybir as mybir
    import concourse.tile as tile
    from concourse import bacc

    key = (cpc, cap, dcap, in_c, f_out, a_fp8)
    if key in _prog_cache:
        return _prog_cache[key]

    kc = in_c // P           # contraction chunks for X @ W
    sch = cap // P           # s-tiles per cluster
    fc = f_out // P          # f chunks (step-2 output partitions)
    hs = sch // 2            # merged step-1 PSUM banks (2 s-tiles each)
    f32 = mybir.dt.float32
    x_dt = mybir.dt.float16
    a_dt = mybir.dt.float8e4 if a_fp8 else x_dt

    nc = bacc.Bacc("TRN2", target_bir_lowering=False, debug=False,
                   num_devices=N_CORES)

    XT = nc.dram_tensor("XT", [P, cpc, kc, cap], x_dt, kind="ExternalInput")
    WT = nc.dram_tensor("WT", [P, kc, f_out], x_dt, kind="ExternalInput")
    AT = nc.dram_tensor("AT", [P, cpc, sch * dcap], a_dt, kind="ExternalInput")
    YT = nc.dram_tensor("YT", [cpc, P, fc, dcap], x_dt, kind="ExternalOutput")

    # prefetch group schedule: small first groups so compute starts early
    gsched = []
    c0 = 0
    for g in [1, 1, 2, 2, 3] + [4] * cpc:
        if c0 >= cpc:
            break
        g = min(g, cpc - c0)
        gsched.append((c0, g))
        c0 += g
    g_of = {}
    for c0, g in gsched:
        for c in range(c0, c0 + g):
            g_of[c] = (c0, g)
    GMAX = max(g for _, g in gsched)

    with tile.TileContext(nc) as tc:
        with (
            tc.tile_pool(name="w", bufs=1) as w_pool,
            tc.tile_pool(name="xt", bufs=4) as xt_pool,
            tc.tile_pool(name="at", bufs=4) as at_pool,
            tc.tile_pool(name="xw", bufs=3 * hs) as xw_pool,
            tc.tile_pool(name="ot", bufs=6) as ot_pool,
            tc.tile_pool(name="ps1", bufs=4, space="PSUM") as ps1_pool,
            tc.tile_pool(name="ps2", bufs=4, space="PSUM") as ps2_pool,
        ):
            scratch = w_pool.tile([P, 512], x_dt)
            nc.gpsimd.memset(scratch[:], 0)
            # first cluster's inputs fan out over the queues in parallel:
            # sync: X0; scalar: W + At0 second half; gpsimd: At0 first half
            xt0 = xt_pool.tile([P, GMAX, kc, cap], x_dt, name="xtt")
            nc.sync.dma_start(xt0[:, :1], XT[:, :1])
            wt = w_pool.tile([P, kc, f_out], x_dt)
            nc.scalar.dma_start(wt[:], WT[:])
            at0 = at_pool.tile([P, GMAX, sch * dcap], a_dt, name="att")
            hd = (sch // 2) * dcap
            nc.gpsimd.dma_start(at0[:, 0, :hd], AT[:, 0, :hd])
            nc.scalar.dma_start(at0[:, 0, hd:], AT[:, 0, hd:])
            # PE p-state warmup while the first loads are in flight
            for _ in range(N_WARM):
                psw = ps2_pool.tile([P, 512], f32, name="ps2t")
                nc.tensor.matmul(psw[:], lhsT=scratch[:, :P],
                                 rhs=scratch[:], start=True, stop=True)

            xt = at = None
            for c in range(cpc):
                c0, g = g_of[c]
                if c == 0:
                    xt, at = xt0, at0
                elif c == c0:
                    xt = xt_pool.tile([P, GMAX, kc, cap], x_dt, name="xtt")
                    nc.sync.dma_start(xt[:, :g], XT[:, c0:c0 + g])
                    at = at_pool.tile([P, GMAX, sch * dcap], a_dt, name="att")
                    nc.scalar.dma_start(at[:, :g], AT[:, c0:c0 + g])
                xoff = (c - c0) * cap
                ci = c - c0

                # step1: Xs @ W, two s-tiles share one PSUM bank
                xwm = []
                for h in range(hs):
                    ps = ps1_pool.tile([P, 2, f_out], f32, name="ps1t")
                    for t in range(2):
                        st = 2 * h + t
                        for k in range(kc):
                            nc.tensor.matmul(
                                ps[:, t],
                                lhsT=xt[:, ci, k, st * P:(st + 1) * P],
                                rhs=wt[:, k],
                                start=(k == 0), stop=(k == kc - 1),
                            )
                    xw = xw_pool.tile([P, 2, f_out], x_dt, name="xwt")
                    if h == 0:
                        nc.scalar.copy(xw[:], ps[:])
                    else:
                        nc.vector.tensor_copy(xw[:], ps[:])
                    xwm.append(xw)

                # step2: Z^T[f, d] = sum_s xws[s, f] * At[s, d]
                ot = ot_pool.tile([P, fc, dcap], x_dt)
                dstep = 512 if c < cpc - 2 else 256
                for f in range(fc):
                    for d0 in range(0, dcap, dstep):
                        dn = min(dstep, dcap - d0)
                        ps = ps2_pool.tile([P, 512], f32, name="ps2t")
                        for st in range(sch):
                            h, t = divmod(st, 2)
                            nc.tensor.matmul(
                                ps[:, :dn],
                                lhsT=xwm[h][:, t, f * P:(f + 1) * P],
                                rhs=at[:, ci, st * dcap + d0:st * dcap + d0 + dn],
                                start=(st == 0), stop=(st == sch - 1),
                            )
                        nc.vector.tensor_copy(ot[:, f, d0:d0 + dn], ps[:, :dn])
                    nc.gpsimd.dma_start(YT[c][:, f], ot[:, f])

    nc.compile()
    _prog_cache[key] = nc
    return nc


def _host_prep(X, W, b, assign, full_ei):
    """Shard + preprocess. Returns (in_maps, a_fp8, gather info)."""
    n, in_c = X.shape
    f_out = W.shape[1]
    src = full_ei[0].astype(np.int64)
    dst = full_ei[1].astype(np.int64)
    a_s = assign[src]
    intra = a_s == assign[dst]
    es, ed = src[intra], dst[intra]

    deg = np.ones(n, np.float32)
    np.add.at(deg, ed, np.float32(1))
    dis = (1.0 / np.sqrt(deg)).astype(np.float32)

    has_edge = np.zeros(N_CLUSTERS, bool)
    has_edge[np.unique(a_s[intra])] = True

    sizes = np.bincount(assign, minlength=N_CLUSTERS)
    cpc = -(-N_CLUSTERS // N_CORES)                 # clusters per core
    cap = max(512, int(-(-sizes.max() // P)) * P)   # padded cluster size (s)
    dcap = int(sizes.max())                         # exact d extent
    sch = cap // P

    starts = np.zeros(N_CLUSTERS + 1, np.int64)
    starts[1:] = np.cumsum(sizes)
    order = np.argsort(assign, kind="stable")
    pos = np.empty(n, np.int64)
    pos[order] = np.arange(n) - starts[assign[order]]

    ctot = cpc * N_CORES
    # At blocks: At[c][s, d] = #edges(s->d) + [s==d]
    At = np.zeros((ctot, cap, dcap), np.uint16)
    np.add.at(At, (assign[es], pos[es], pos[ed]), 1)
    At[assign, pos, pos] += 1
    a_fp8 = int(At.max()) <= 16    # integers <= 16 are exact in e4m3

    if a_fp8:
        import concourse.mybir as mybir
        a_np = mybir.dt.np(mybir.dt.float8e4)
    else:
        a_np = np.float16
    # [c, s, d] -> [p, c, st*dcap + d] so each partition row is contiguous
    At_send = np.ascontiguousarray(
        At.astype(a_np).reshape(ctot, sch, P, dcap).transpose(2, 0, 1, 3)
    ).reshape(P, ctot, sch * dcap)

    # pre-scaled X, padded per cluster, partition-major with 2KB rows:
    # XT[p, c, k, j] = Xs[c, j, k*P + p]
    Xs = X.astype(np.float32) * dis[:, None]
    Xp = np.zeros((ctot, cap, in_c), np.float32)
    Xp[assign, pos] = Xs
    kc = in_c // P
    XT_all = np.ascontiguousarray(
        Xp.transpose(2, 0, 1).reshape(kc, P, ctot, cap)
        .transpose(1, 2, 0, 3)).astype(np.float16)

    WT_send = np.ascontiguousarray(
        W.astype(np.float32).reshape(kc, P, f_out).transpose(1, 0, 2)
    ).astype(np.float16)

    in_maps = []
    for i in range(N_CORES):
        in_maps.append({
            "XT": np.ascontiguousarray(XT_all[:, i * cpc:(i + 1) * cpc]),
            "WT": WT_send,
            "AT": np.ascontiguousarray(At_send[:, i * cpc:(i + 1) * cpc]),
        })
    return in_maps, a_fp8, (cpc, cap, dcap, has_edge, pos, dis)


def _run(inputs, trace=False, tmpdir=None):
    from concourse.bass_utils import run_bass_kernel_spmd

    X = np.asarray(inputs["X"], np.float32)
    W = np.asarray(inputs["W"], np.float32)
    b = np.asarray(inputs["b"], np.float32)
    assign = np.asarray(inputs["assign"])
    full_ei = np.asarray(inputs["full_ei"])

    n, in_c = X.shape
    f_out = W.shape[1]
    in_maps, a_fp8, (cpc, cap, dcap, has_edge, pos, dis) = _host_prep(
        X, W, b, assign, full_ei)
    nc = _build_program(cpc, cap, dcap, in_c, f_out, a_fp8)

    res = run_bass_kernel_spmd(
        nc, in_maps, core_ids=list(range(N_CORES)),
        trace=trace, tmpdir=tmpdir,
    )
    # YT: [core][cpc, P, fc, dcap]; Y[n, fi*P + p] = YT[core, lc, p, fi, pos]
    YTdev = np.stack([res.results[i]["YT"] for i in range(N_CORES)])
    if YTdev.dtype != np.float32:
        YTdev = YTdev.astype(np.float32)
    fc = f_out // P
    Yt = YTdev.transpose(0, 1, 3, 2, 4).reshape(N_CORES, cpc, f_out, dcap)

    c = assign.astype(np.int64)
    core = c // cpc
    lc = c % cpc
    Y = Yt[core, lc, :, pos]
    Y *= dis[:, None]
    Y += b[None, :].astype(np.float32)
    miss = ~has_edge[c]
    if miss.any():
        Y[miss] = X[miss]
    return Y, res


def kernel(**inputs) -> np.ndarray:
    Y, _ = _run(inputs)
    return Y


# revision 15
# speedup vs baseline: 1.0333x; 1.0333x over previous
"""ClusterGCN layer on 8 TRN2 NeuronCores.

Math per cluster c (only intra-cluster edges matter):
    Y_c = D^-1/2 (A_c + I) D^-1/2 X_c W + b,  D = intra-degree + 1.
Host pre-scales X rows by dis = rsqrt(deg) and ships integer edge
counts At (exact in fp8e4 for counts <= 16); device computes
    xws = Xs @ W                      (nodes on partitions)
    Z^T[f, d] = sum_s xws[s, f] * At_c[s, d]
host applies dis[d] + bias on gather and passes through clusters with
no intra edges.

Device pipeline per cluster (all matmuls fp16 on the PE):
  step1: 2 PSUM banks, each accumulating two 128-node s-tiles of
         Xs @ W; drained by scalar (bank 0) and vector (bank 1) casts.
  step2: fc x d-chunk matmuls vs fp8 At (moving), drained by vector.
DRAM layouts are partition-major so every DMA row is one long run:
  XT [P, kc, cpc*cap]   AT [P, cpc, sch*dcap]   YT [cpc, P, fc, dcap]
Queues: sync = W/X/At loads (groups of 1,1,2,2,3,4 clusters so compute
starts early), gpsimd = YT stores (stores never block upcoming loads).
Five dummy matmuls on a zeroed scratch tile warm the PE out of its
low/mid p-states (0.65/1.2 GHz) while the first loads are in flight.
"""

import numpy as np

N_CORES = 8
N_CLUSTERS = 100
P = 128
N_WARM = 7

_prog_cache: dict = {}


def _build_program(cpc: int, cap: int, dcap: int, in_c: int, f_out: int,
                   a_fp8: bool):
    import concourse.mybir as mybir
    import concourse.tile as tile
    from concourse import bacc

    key = (cpc, cap, dcap, in_c, f_out, a_fp8)
    if key in _prog_cache:
        return _prog_cache[key]

    kc = in_c // P           # contraction chunks for X @ W
    sch = cap // P           # s-tiles per cluster
    fc = f_out // P          # f chunks (step-2 output partitions)
    hs = sch // 2            # merged step-1 PSUM banks (2 s-tiles each)
    f32 = mybir.dt.float32
    x_dt = mybir.dt.float16
    a_dt = mybir.dt.float8e4 if a_fp8 else x_dt

    nc = bacc.Bacc("TRN2", target_bir_lowering=False, debug=False,
                   num_devices=N_CORES)

    XT = nc.dram_tensor("XT", [P, cpc, kc, cap], x_dt, kind="ExternalInput")
    WT = nc.dram_tensor("WT", [P, kc, f_out], x_dt, kind="ExternalInput")
    AT = nc.dram_tensor("AT", [P, cpc, sch * dcap], a_dt, kind="ExternalInput")
    YT = nc.dram_tensor("YT", [cpc, P, fc, dcap], x_dt, kind="ExternalOutput")

    # prefetch group schedule: small first groups so compute starts early
    gsched = []
    c0 = 0
    for g in [1, 1, 2, 2, 3] + [4] * cpc:
        if c0 >= cpc:
            break
        g = min(g, cpc - c0)
        gsched.append((c0, g))
        c0 += g
    g_of = {}
    for c0, g in gsched:
        for c in range(c0, c0 + g):
            g_of[c] = (c0, g)
    GMAX = max(g for _, g in gsched)

    with tile.TileContext(nc) as tc:
        with (
            tc.tile_pool(name="w", bufs=1) as w_pool,
            tc.tile_pool(name="xt", bufs=4) as xt_pool,
            tc.tile_pool(name="at", bufs=4) as at_pool,
            tc.tile_pool(name="xw", bufs=3 * hs) as xw_pool,
            tc.tile_pool(name="ot", bufs=6) as ot_pool,
            tc.tile_pool(name="ps1", bufs=4, space="PSUM") as ps1_pool,
            tc.tile_pool(name="ps2", bufs=4, space="PSUM") as ps2_pool,
        ):
            scratch = w_pool.tile([P, 512], x_dt)
            nc.gpsimd.memset(scratch[:], 0)
            # first cluster's inputs fan out over the queues in parallel:
            # sync: X0; scalar: W + At0 second half; gpsimd: At0 first half
            xt0 = xt_pool.tile([P, GMAX, kc, cap], x_dt, name="xtt")
            nc.sync.dma_start(xt0[:, :1], XT[:, :1])
            wt = w_pool.tile([P, kc, f_out], x_dt)
            nc.scalar.dma_start(wt[:], WT[:])
            at0 = at_pool.tile([P, GMAX, sch * dcap], a_dt, name="att")
            hd = (sch // 2) * dcap
            nc.gpsimd.dma_start(at0[:, 0, :hd], AT[:, 0, :hd])
            nc.scalar.dma_start(at0[:, 0, hd:], AT[:, 0, hd:])
            # PE p-state warmup while the first loads are in flight
            for _ in range(N_WARM):
                psw = ps2_pool.tile([P, 512], f32, name="ps2t")
                nc.tensor.matmul(psw[:], lhsT=scratch[:, :P],
                                 rhs=scratch[:], start=True, stop=True)

            xt = at = None
            for c in range(cpc):
                c0, g = g_of[c]
                if c == 0:
                    xt, at = xt0, at0
                elif c == c0:
                    xt = xt_pool.tile([P, GMAX, kc, cap], x_dt, name="xtt")
                    nc.sync.dma_start(xt[:, :g], XT[:, c0:c0 + g])
                    at = at_pool.tile([P, GMAX, sch * dcap], a_dt, name="att")
                    nc.scalar.dma_start(at[:, :g], AT[:, c0:c0 + g])
                xoff = (c - c0) * cap
                ci = c - c0

                # step1: Xs @ W, two s-tiles share one PSUM bank
                xwm = []
                for h in range(hs):
                    ps = ps1_pool.tile([P, 2, f_out], f32, name="ps1t")
                    for t in range(2):
                        st = 2 * h + t
                        for k in range(kc):
                            nc.tensor.matmul(
                                ps[:, t],
                                lhsT=xt[:, ci, k, st * P:(st + 1) * P],
                                rhs=wt[:, k],
                                start=(k == 0), stop=(k == kc - 1),
                            )
                    xw = xw_pool.tile([P, 2, f_out], x_dt, name="xwt")
                    if h == 0:
                        nc.scalar.copy(xw[:], ps[:])
                    else:
                        nc.vector.tensor_copy(xw[:], ps[:])
                    xwm.append(xw)

                # step2: Z^T[f, d] = sum_s xws[s, f] * At[s, d]
                ot = ot_pool.tile([P, fc, dcap], x_dt)
                dstep = 512 if c < cpc - 2 else 256
                for f in range(fc):
                    for d0 in range(0, dcap, dstep):
                        dn = min(dstep, dcap - d0)
                        ps = ps2_pool.tile([P, 512], f32, name="ps2t")
                        for st in range(sch):
                            h, t = divmod(st, 2)
                            nc.tensor.matmul(
                                ps[:, :dn],
                                lhsT=xwm[h][:, t, f * P:(f + 1) * P],
                                rhs=at[:, ci, st * dcap + d0:st * dcap + d0 + dn],
                                start=(st == 0), stop=(st == sch - 1),
                            )
                        nc.vector.tensor_copy(ot[:, f, d0:d0 + dn], ps[:, :dn])
                    nc.scalar.dma_start(YT[c][:, f], ot[:, f])

    nc.compile()
    _prog_cache[key] = nc
    return nc


def _host_prep(X, W, b, assign, full_ei):
    """Shard + preprocess. Returns (in_maps, a_fp8, gather info)."""
    n, in_c = X.shape
    f_out = W.shape[1]
    src = full_ei[0].astype(np.int64)
    dst = full_ei[1].astype(np.int64)
    a_s = assign[src]
    intra = a_s == assign[dst]
    es, ed = src[intra], dst[intra]

    deg = np.ones(n, np.float32)
    np.add.at(deg, ed, np.float32(1))
    dis = (1.0 / np.sqrt(deg)).astype(np.float32)

    has_edge = np.zeros(N_CLUSTERS, bool)
    has_edge[np.unique(a_s[intra])] = True

    sizes = np.bincount(assign, minlength=N_CLUSTERS)
    cpc = -(-N_CLUSTERS // N_CORES)                 # clusters per core
    cap = max(512, int(-(-sizes.max() // P)) * P)   # padded cluster size (s)
    dcap = int(sizes.max())                         # exact d extent
    sch = cap // P

    starts = np.zeros(N_CLUSTERS + 1, np.int64)
    starts[1:] = np.cumsum(sizes)
    order = np.argsort(assign, kind="stable")
    pos = np.empty(n, np.int64)
    pos[order] = np.arange(n) - starts[assign[order]]

    ctot = cpc * N_CORES
    # At blocks: At[c][s, d] = #edges(s->d) + [s==d]
    At = np.zeros((ctot, cap, dcap), np.uint16)
    np.add.at(At, (assign[es], pos[es], pos[ed]), 1)
    At[assign, pos, pos] += 1
    a_fp8 = int(At.max()) <= 16    # integers <= 16 are exact in e4m3

    if a_fp8:
        import concourse.mybir as mybir
        a_np = mybir.dt.np(mybir.dt.float8e4)
    else:
        a_np = np.float16
    # [c, s, d] -> [p, c, st*dcap + d] so each partition row is contiguous
    At_send = np.ascontiguousarray(
        At.astype(a_np).reshape(ctot, sch, P, dcap).transpose(2, 0, 1, 3)
    ).reshape(P, ctot, sch * dcap)

    # pre-scaled X, padded per cluster, partition-major with 2KB rows:
    # XT[p, c, k, j] = Xs[c, j, k*P + p]
    Xs = X.astype(np.float32) * dis[:, None]
    Xp = np.zeros((ctot, cap, in_c), np.float32)
    Xp[assign, pos] = Xs
    kc = in_c // P
    XT_all = np.ascontiguousarray(
        Xp.transpose(2, 0, 1).reshape(kc, P, ctot, cap)
        .transpose(1, 2, 0, 3)).astype(np.float16)

    WT_send = np.ascontiguousarray(
        W.astype(np.float32).reshape(kc, P, f_out).transpose(1, 0, 2)
    ).astype(np.float16)

    in_maps = []
    for i in range(N_CORES):
        in_maps.append({
            "XT": np.ascontiguousarray(XT_all[:, i * cpc:(i + 1) * cpc]),
            "WT": WT_send,
            "AT": np.ascontiguousarray(At_send[:, i * cpc:(i + 1) * cpc]),
        })
    return in_maps, a_fp8, (cpc, cap, dcap, has_edge, pos, dis)


def _run(inputs, trace=False, tmpdir=None):
    from concourse.bass_utils import run_bass_kernel_spmd

    X = np.asarray(inputs["X"], np.float32)
    W = np.asarray(inputs["W"], np.float32)
    b = np.asarray(inputs["b"], np.float32)
    assign = np.asarray(inputs["assign"])
    full_ei = np.asarray(inputs["full_ei"])

    n, in_c = X.shape
    f_out = W.shape[1]
    in_maps, a_fp8, (cpc, cap, dcap, has_edge, pos, dis) = _host_prep(
        X, W, b, assign, full_ei)
    nc = _build_program(cpc, cap, dcap, in_c, f_out, a_fp8)

    res = run_bass_kernel_spmd(
        nc, in_maps, core_ids=list(range(N_CORES)),
        trace=trace, tmpdir=tmpdir,
    )
    # YT: [core][cpc, P, fc, dcap]; Y[n, fi*P + p] = YT[core, lc, p, fi, pos]
    YTdev = np.stack([res.results[i]["YT"] for i in range(N_CORES)])
    if YTdev.dtype != np.float32:
        YTdev = YTdev.astype(np.float32)
    fc = f_out // P
    Yt = YTdev.transpose(0, 1, 3, 2, 4).reshape(N_CORES, cpc, f_out, dcap)

    c = assign.astype(np.int64)
    core = c // cpc
    lc = c % cpc
    Y = Yt[core, lc, :, pos]
    Y *= dis[:, None]
    Y += b[None, :].astype(np.float32)
    miss = ~has_edge[c]
    if miss.any():
        Y[miss] = X[miss]
    return Y, res


def kernel(**inputs) -> np.ndarray:
    Y, _ = _run(inputs)
    return Y
